# revision 1
# baseline (speedup 1.0000x reference)
"""TRN2 Bass kernel for nn_MultiHeadMemory (H=16, M=1024, D=512, O=512, N=16384).

Strategy (8 NeuronCores):
  Stage A (head-parallel, 2 heads/core): per head h compute
     expkeyT[o,m] = exp(mems_h @ Wk_h^T + bk_h)^T          (unnormalized keys, transposed)
     svec[m]      = 1 / sum_o expkey[m,o]                  (key-softmax normalizer)
     val2[m,:]    = (mems_h @ Wv_h^T + bv_h) @ Wfh^T (+bf) (final Linear folded per head)
  then AllGather the (expkeyT, val2, svec) payloads across cores.
  Stage C (N-parallel, 2048 query rows/core): for every head h
     attT = expkeyT_h^T-contraction with kT (PE), eatt = exp(svec_h * attT) (ACT),
     out += (eatt^T @ val2_h) / (eatt^T @ 1)               (PE + DVE normalize-accumulate)
  The final Linear never materializes: x @ Wf^T == sum_h att_h @ (val_h @ Wfh^T),
  and bf is folded into head 0's val2 (attention rows sum to 1).
  Matmuls run in float32r (full PE rate); accumulation fp32 in PSUM.
"""

import numpy as np

H, M, D, O, N = 16, 1024, 512, 512, 16384
NCORES = 8
HPC = H // NCORES          # heads per core
NS = N // NCORES           # query rows per core

EK_SZ = O * M              # expkeyT floats per head
V2_SZ = M * O              # val2 floats per head
SV_SZ = M                  # svec floats per head
PAYLOAD = EK_SZ + V2_SZ + SV_SZ


def build_nc(ns=NS, rep=1, mock_cc=False, c_bf16=False):
    """Build + compile the SPMD Bass program (same program on all 8 cores)."""
    from contextlib import ExitStack
    import concourse.tile as tile
    from concourse import bacc, mybir, masks

    f32 = mybir.dt.float32
    fr = mybir.dt.float32r
    cdt = mybir.dt.bfloat16 if c_bf16 else fr
    AF = mybir.ActivationFunctionType

    OT, DTL, MT = O // 128, D // 128, M // 128      # 4, 4, 8
    NT = ns // 128
    NCH = ns // 512

    nc = bacc.Bacc("TRN2", target_bir_lowering=False, debug=False,
                   num_devices=NCORES)

    k_in = nc.dram_tensor("k", [ns, O], f32, kind="ExternalInput")
    mems_in = nc.dram_tensor("mems", [HPC, M, D], f32, kind="ExternalInput")
    wk_in = nc.dram_tensor("Wk", [HPC, O, D], f32, kind="ExternalInput")
    bk_in = nc.dram_tensor("bk", [HPC, O], fr, kind="ExternalInput")
    wv_in = nc.dram_tensor("Wv", [HPC, O, D], f32, kind="ExternalInput")
    bv_in = nc.dram_tensor("bv", [HPC, O], f32, kind="ExternalInput")
    wf_in = nc.dram_tensor("Wfh", [HPC, O, O], f32, kind="ExternalInput")
    bf_in = nc.dram_tensor("bf", [HPC, O], fr, kind="ExternalInput")
    out_ext = nc.dram_tensor("out", [ns, O], f32, kind="ExternalOutput")

    def b(ap):  # float32r view for matmul operands
        return ap.bitcast(fr)

    with tile.TileContext(nc, pool_alloc_mode="queue") as tc, ExitStack() as octx:
        dram_pool = octx.enter_context(
            tc.tile_pool(name="dram", bufs=1, space="DRAM"))
        const_pool = octx.enter_context(tc.tile_pool(name="const", bufs=1))
        ident = const_pool.tile([128, 128], f32)
        masks.make_identity(nc, ident[:])
        ones_col = const_pool.tile([128, 2], cdt)
        ones_col_f32 = const_pool.tile([128, 2], f32)
        nc.gpsimd.memset(ones_col_f32[:], 1.0)
        nc.scalar.copy(ones_col[:], ones_col_f32[:])
        ones_row = const_pool.tile([1, 128], fr)
        ones_row_f32 = const_pool.tile([1, 128], f32)
        nc.gpsimd.memset(ones_row_f32[:], 1.0)
        nc.scalar.copy(ones_row[:], ones_row_f32[:])

        kt_pool = octx.enter_context(tc.tile_pool(name="kt", bufs=1))
        acc_pool = octx.enter_context(tc.tile_pool(name="acc", bufs=1))

        for r in range(rep):
            agg_ins = [dram_pool.tile([PAYLOAD], cdt, tag=f"agg_in{r}_{j}",
                                      name=f"agg_in{r}_{j}")
                       for j in range(HPC)]
            agg_outs = [dram_pool.tile([NCORES * PAYLOAD], cdt,
                                       tag=f"agg_out{r}_{j}",
                                       name=f"agg_out{r}_{j}",
                                       addr_space="Shared")
                        for j in range(HPC)]
            # ============ Stage A: per-local-head key/val precompute ========
            with ExitStack() as actx:
                small = actx.enter_context(tc.tile_pool(name=f"small{r}", bufs=2))
                tp_ps = actx.enter_context(
                    tc.tile_pool(name=f"tp_ps{r}", bufs=4, space="PSUM"))
                mm_ps = actx.enter_context(
                    tc.tile_pool(name=f"mm_ps{r}", bufs=2, space="PSUM"))

                ev_cnt = [0]

                def evac(dst_ap, src_ap):
                    eng = nc.scalar if (ev_cnt[0] % 2 == 0) else nc.vector
                    ev_cnt[0] += 1
                    if eng is nc.scalar:
                        eng.copy(dst_ap, src_ap)
                    else:
                        eng.tensor_copy(dst_ap, src_ap)

                def transpose128(dst_ap, src_ap):
                    p = tp_ps.tile([128, 128], f32, tag="tp", name="tp_ps_t")
                    nc.tensor.transpose(p[:], src_ap, ident[:])
                    evac(dst_ap, p[:])

                def load_transposed(src_dram, nrow_t, ncol_t, nm):
                    # transposed dest allocated FIRST (outlives the staging load)
                    tt, ftt = tc.tile([128, ncol_t, nrow_t * 128], fr,
                                      name=nm + "T")
                    ld, fld = tc.tile([128, nrow_t, ncol_t * 128], f32, name=nm)
                    nc.sync.dma_start(
                        ld[:], src_dram.rearrange("(a p) d -> p a d", p=128))
                    for a in range(nrow_t):
                        for c in range(ncol_t):
                            transpose128(
                                tt[:, c, a * 128:(a + 1) * 128],
                                ld[:, a, c * 128:(c + 1) * 128])
                    fld()
                    return tt, ftt

                for j in range(HPC):
                    bk_sb = small.tile([1, O], fr, tag="bk_ld", name="bk_sb")
                    nc.sync.dma_start(
                        bk_sb[:], bk_in[j].rearrange("(a o) -> a o", a=1))
                    bf_sb = small.tile([1, O], fr, tag="bf_ld", name="bf_sb")
                    nc.sync.dma_start(
                        bf_sb[:], bf_in[j].rearrange("(a o) -> a o", a=1))
                    bv_sb = small.tile([128, OT], f32, tag="bv_ld", name="bv_sb")
                    nc.sync.dma_start(
                        bv_sb[:], bv_in[j].rearrange("(t p) -> p t", p=128))

                    # ---- memsT [d, m] (lives until valT is computed)
                    memsT, f_memsT = load_transposed(mems_in[j], MT, DTL, "mems")

                    # ---- key logits + exp (+ row sums)
                    expkey, f_expkey = tc.tile([128, MT, O], f32, name="expkey")
                    wkT, f_wkT = load_transposed(wk_in[j], OT, DTL, "wk")
                    ksum = small.tile([128, MT], f32, tag="ksum", name="ksum")
                    for mt in range(MT):
                        pk = mm_ps.tile([128, O], f32, tag="mm", name="pk")
                        for dk in range(DTL):
                            nc.tensor.matmul(
                                pk[:],
                                (memsT[:, dk, mt * 128:(mt + 1) * 128]),
                                (wkT[:, dk, :]),
                                start=(dk == 0), stop=False)
                        nc.tensor.matmul(
                            pk[:], (ones_row[:1, :]), (bk_sb[:1, :]),
                            start=False, stop=True)
                        nc.scalar.activation(
                            expkey[:, mt, :], pk[:], AF.Exp,
                            accum_out=ksum[:, mt:mt + 1])
                    f_wkT()
                    svec = small.tile([128, MT], f32, tag="svec", name="svec")
                    nc.vector.reciprocal(svec[:], ksum[:])

                    # ---- expkeyT -> DMA out
                    ekT, f_ekT = tc.tile([128, OT, M], cdt, name="ekT")
                    for mt in range(MT):
                        for ot in range(OT):
                            transpose128(
                                ekT[:, ot, mt * 128:(mt + 1) * 128],
                                expkey[:, mt, ot * 128:(ot + 1) * 128])
                    nc.sync.dma_start(
                        agg_ins[j][0:EK_SZ].rearrange(
                            "(ot p m) -> p ot m", ot=OT, p=128), ekT[:])
                    f_ekT()
                    f_expkey()

                    # ---- valT [o, m] with bias bv
                    valT, f_valT = tc.tile([128, DTL, M], fr, name="valT")
                    wvT, f_wvT = load_transposed(wv_in[j], OT, DTL, "wv")
                    for ot in range(OT):
                        for mc in range(M // 512):
                            pv = mm_ps.tile([128, 512], f32, tag="mm", name="pv")
                            for dk in range(DTL):
                                nc.tensor.matmul(
                                    pv[:],
                                    (wvT[:, dk, ot * 128:(ot + 1) * 128]),
                                    (memsT[:, dk, mc * 512:(mc + 1) * 512]),
                                    start=(dk == 0), stop=(dk == DTL - 1))
                            nc.scalar.add(
                                valT[:, ot, mc * 512:(mc + 1) * 512], pv[:],
                                bv_sb[:, ot:ot + 1])
                    f_wvT()

                    # ---- val2 [m, oo] = valT^T @ WfhT (+ bf)
                    val2, f_val2 = tc.tile([128, MT, O], cdt, name="val2")
                    wfT, f_wfT = load_transposed(wf_in[j], OT, OT, "wf")
                    for mt in range(MT):
                        p2 = mm_ps.tile([128, O], f32, tag="mm", name="p2")
                        for ot in range(OT):
                            nc.tensor.matmul(
                                p2[:],
                                (valT[:, ot, mt * 128:(mt + 1) * 128]),
                                (wfT[:, ot, :]),
                                start=(ot == 0), stop=False)
                        nc.tensor.matmul(
                            p2[:], (ones_row[:1, :]), (bf_sb[:1, :]),
                            start=False, stop=True)
                        evac(val2[:, mt, :], p2[:])
                    off = EK_SZ
                    nc.sync.dma_start(
                        agg_ins[j][off:off + V2_SZ].rearrange(
                            "(mt p f) -> p mt f", mt=MT, p=128), val2[:])
                    svec_c = small.tile([128, MT], cdt, tag="svec_c",
                                        name="svec_c")
                    nc.scalar.copy(svec_c[:], svec[:])
                    off = EK_SZ + V2_SZ
                    nc.sync.dma_start(
                        agg_ins[j][off:off + SV_SZ].rearrange(
                            "(p t) -> p t", p=128), svec_c[:])
                    f_wfT()
                    f_val2()
                    f_valT()
                    f_memsT()
                    if not mock_cc:
                        nc.gpsimd.collective_compute(
                            "AllGather", mybir.AluOpType.bypass,
                            replica_groups=[list(range(NCORES))],
                            ins=[agg_ins[j][:]], outs=[agg_outs[j][:]])

                # ============ kT: transpose this core's k slice ============
                kT = kt_pool.tile([128, OT, ns], cdt, tag="kT", name="kT")
                for ng in range(NT // 4):
                    k_sb = small.tile([128, 4, O], f32, tag="k_ld", name="k_sb")
                    nc.sync.dma_start(
                        k_sb[:],
                        k_in[ng * 512:(ng + 1) * 512, :].rearrange(
                            "(nt p) o -> p nt o", p=128))
                    for nt in range(4):
                        for ot in range(OT):
                            transpose128(
                                kT[:, ot, (ng * 4 + nt) * 128:(ng * 4 + nt + 1) * 128],
                                k_sb[:, nt, ot * 128:(ot + 1) * 128])

            # ============ Stage C: attention over all heads ============
            acc = acc_pool.tile([128, NT, O], f32, tag="acc")
            with ExitStack() as cctx:
                h_ld = cctx.enter_context(tc.tile_pool(name=f"h_ld{r}", bufs=2))
                e_sb = cctx.enter_context(tc.tile_pool(name=f"e_sb{r}", bufs=2))
                v_sb = cctx.enter_context(tc.tile_pool(name=f"v_sb{r}", bufs=2))
                att_ps = cctx.enter_context(
                    tc.tile_pool(name=f"att_ps{r}", bufs=4, space="PSUM"))
                o_ps = cctx.enter_context(
                    tc.tile_pool(name=f"o_ps{r}", bufs=2, space="PSUM"))
                rs_ps = cctx.enter_context(
                    tc.tile_pool(name=f"rs_ps{r}", bufs=2, space="PSUM"))

                for hidx in range(H):
                    j, cc = hidx // NCORES, hidx % NCORES
                    if mock_cc:
                        ek_src, base = agg_ins[j], 0
                    else:
                        ek_src, base = agg_outs[j], cc * PAYLOAD
                    ekt_h = h_ld.tile([128, OT, M], cdt, tag="ekt_h")
                    nc.sync.dma_start(
                        ekt_h[:],
                        ek_src[base:base + EK_SZ].rearrange(
                            "(ot p m) -> p ot m", ot=OT, p=128))
                    val2_h = h_ld.tile([128, MT, O], cdt, tag="val2_h")
                    nc.sync.dma_start(
                        val2_h[:],
                        ek_src[base + EK_SZ:base + EK_SZ + V2_SZ].rearrange(
                            "(mt p f) -> p mt f", mt=MT, p=128))
                    svec_hc = h_ld.tile([128, MT], cdt, tag="svec_hc")
                    nc.sync.dma_start(
                        svec_hc[:],
                        ek_src[base + EK_SZ + V2_SZ:base + PAYLOAD].rearrange(
                            "(p t) -> p t", p=128))
                    svec_h = h_ld.tile([128, MT], f32, tag="svec_h")
                    nc.vector.tensor_copy(svec_h[:], svec_hc[:])

                    for c in range(NCH):
                        eatt = e_sb.tile([128, MT, 512], cdt, tag="eatt")
                        for mt in range(MT):
                            pa = att_ps.tile([128, 512], f32, tag="att")
                            for ot in range(OT):
                                nc.tensor.matmul(
                                    pa[:],
                                    (ekt_h[:, ot, mt * 128:(mt + 1) * 128]),
                                    (kT[:, ot, c * 512:(c + 1) * 512]),
                                    start=(ot == 0), stop=(ot == OT - 1))
                            nc.scalar.activation(
                                eatt[:, mt, :], pa[:], AF.Exp,
                                scale=svec_h[:, mt:mt + 1])
                        for nt in range(4):
                            po = o_ps.tile([128, O], f32, tag="o")
                            prs = rs_ps.tile([128, 2], f32, tag="rs")
                            for mt in range(MT):
                                nc.tensor.matmul(
                                    po[:],
                                    (eatt[:, mt, nt * 128:(nt + 1) * 128]),
                                    (val2_h[:, mt, :]),
                                    start=(mt == 0), stop=(mt == MT - 1))
                            for mt in range(MT):
                                nc.tensor.matmul(
                                    prs[:],
                                    (eatt[:, mt, nt * 128:(nt + 1) * 128]),
                                    (ones_col[:]),
                                    start=(mt == 0), stop=(mt == MT - 1))
                            rec = v_sb.tile([128, 1], f32, tag="rec")
                            nc.vector.reciprocal(rec[:], prs[:, :1])
                            gnt = c * 4 + nt
                            if hidx == 0:
                                nc.vector.tensor_scalar_mul(
                                    acc[:, gnt, :], po[:], rec[:, :1])
                            else:
                                tmp = v_sb.tile([128, O], f32, tag="tmp")
                                nc.vector.tensor_scalar_mul(
                                    tmp[:], po[:], rec[:, :1])
                                nc.vector.tensor_add(
                                    acc[:, gnt, :], acc[:, gnt, :], tmp[:])

            nc.sync.dma_start(
                out_ext[:, :].rearrange("(nt p) o -> p nt o", p=128), acc[:])

    nc.compile()
    return nc


# ----------------------------------------------------------------------------
# Host-side execution: persistent jitted 8-core dispatch (axon/PJRT).
# ----------------------------------------------------------------------------
_EXEC_CACHE = {}


def _get_exec(ns=NS, rep=1, c_bf16=False):
    key = (ns, rep, c_bf16)
    if key in _EXEC_CACHE:
        return _EXEC_CACHE[key]

    import jax
    import numpy as _np
    from jax.sharding import Mesh, PartitionSpec
    from jax.experimental.shard_map import shard_map
    from concourse import mybir
    from concourse.bass2jax import (_bass_exec_p, install_neuronx_cc_hook,
                                    partition_id_tensor)

    nc = build_nc(ns=ns, rep=rep, c_bf16=c_bf16)
    # surface walrus/compile errors (PJRT swallows python hook exceptions)
    from concourse import bass2jax as _b2j
    if not getattr(_b2j, "_hook_wrapped", False):
        _orig = _b2j.neuronx_cc_hook

        def _wrapped(*a, **kw):
            try:
                return _orig(*a, **kw)
            except BaseException:
                import traceback
                traceback.print_exc()
                raise
        _b2j.neuronx_cc_hook = _wrapped
        _b2j._hook_wrapped = True
    install_neuronx_cc_hook()

    partition_name = (nc.partition_id_tensor.name
                      if nc.partition_id_tensor else None)
    in_names, out_names, out_avals, zero_outs = [], [], [], []
    for alloc in nc.m.functions[0].allocations:
        if not isinstance(alloc, mybir.MemoryLocationSet):
            continue
        name = alloc.memorylocations[0].name
        if alloc.kind == "ExternalInput":
            if name != partition_name:
                in_names.append(name)
        elif alloc.kind == "ExternalOutput":
            out_names.append(name)
            out_avals.append(jax.core.ShapedArray(
                tuple(alloc.tensor_shape), mybir.dt.np(alloc.dtype)))
            zero_outs.append(_np.zeros(tuple(alloc.tensor_shape),
                                       mybir.dt.np(alloc.dtype)))
    names_all = list(in_names) + list(out_names)
    if partition_name is not None:
        names_all.append(partition_name)

    def _body(*args):
        operands = list(args)
        if partition_name is not None:
            operands.append(partition_id_tensor())
        return tuple(_bass_exec_p.bind(
            *operands, out_avals=tuple(out_avals), in_names=tuple(names_all),
            out_names=tuple(out_names), lowering_input_output_aliases=(),
            sim_require_finite=True, sim_require_nnan=True, nc=nc))

    devices = jax.devices()[:NCORES]
    mesh = Mesh(_np.asarray(devices), ("core",))
    n_args = len(in_names) + len(out_names)
    fn = jax.jit(
        shard_map(_body, mesh=mesh,
                  in_specs=(PartitionSpec("core"),) * n_args,
                  out_specs=(PartitionSpec("core"),) * len(out_names),
                  check_rep=False),
        keep_unused=True)

    exec_info = {
        "fn": fn, "in_names": in_names, "out_names": out_names,
        "zero_outs": zero_outs, "nc": nc, "mesh": mesh,
    }
    _EXEC_CACHE[key] = exec_info
    return exec_info


def make_in_maps(k, mems, Wk, bk, Wv, bv, Wf, bf):
    """Shard full inputs into per-core input dicts."""
    c32 = lambda x: np.ascontiguousarray(np.asarray(x, dtype=np.float32))
    k, mems, Wk, bk, Wv, bv, Wf, bf = map(c32, (k, mems, Wk, bk, Wv, bv, Wf, bf))
    in_maps = []
    for r in range(NCORES):
        h0 = r * HPC
        wfh = np.stack([
            np.ascontiguousarray(Wf[:, (h0 + j) * O:(h0 + j + 1) * O])
            for j in range(HPC)])
        bf_eff = np.zeros((HPC, O), dtype=np.float32)
        if r == 0:
            bf_eff[0] = bf
        in_maps.append({
            "k": k[r * NS:(r + 1) * NS],
            "mems": mems[h0:h0 + HPC],
            "Wk": Wk[h0:h0 + HPC], "bk": bk[h0:h0 + HPC],
            "Wv": Wv[h0:h0 + HPC], "bv": bv[h0:h0 + HPC],
            "Wfh": wfh, "bf": bf_eff,
        })
    return in_maps


def run_on_hw(in_maps, rep=1, c_bf16=False):
    """Run the SPMD program; returns full [N, O] output."""
    import jax
    import jax.numpy as jnp
    from jax.sharding import NamedSharding, PartitionSpec
    ex = _get_exec(ns=NS, rep=rep, c_bf16=c_bf16)
    sh = NamedSharding(ex["mesh"], PartitionSpec("core"))
    args = [
        jax.device_put(np.concatenate([m[name] for m in in_maps], axis=0), sh)
        for name in ex["in_names"]]
    zeros = [
        jnp.zeros((NCORES * z.shape[0], *z.shape[1:]), z.dtype,
                  device=sh)
        for z in ex["zero_outs"]]
    outs = ex["fn"](*args, *zeros)
    out = np.asarray(outs[ex["out_names"].index("out")])
    return out


def kernel(**inputs):
    in_maps = make_in_maps(
        inputs["k"], inputs["mems"], inputs["Wk"], inputs["bk"],
        inputs["Wv"], inputs["bv"], inputs["Wf"], inputs["bf"])
    return run_on_hw(in_maps, rep=1)



# revision 6
# speedup vs baseline: 6.5990x; 6.5990x over previous
"""TRN2 Bass kernel for nn_MultiHeadMemory (H=16, M=1024, D=512, O=512, N=16384).

Strategy (8 NeuronCores):
  Host prep: mems/Wk/Wv/Wfh are passed pre-transposed (d-major / o-major) so
  stage A needs no on-device transposes; bk is shifted by -2 so unnormalized
  keys fit comfortably in fp8-e4m3 range (softmax is shift-invariant; svec is
  computed from the stored values so the shift cancels).

  Stage A (head-parallel, 2 heads/core): per head h, in [o, m] orientation:
     ekT[o,m]  = exp(WkT^T-contract memsT + bk - 2)   -> fp8 payload, no transpose
     svec[m]   = 1 / sum_o ekT[o,m]                   (tiny PE matmuls w/ ones)
     val2[m,:] = (mems_h @ Wv_h^T + bv_h) @ Wfh^T (+bf) -> bf16 payload
  then AllGather (ekT fp8, svec f32, val2 bf16) across cores.

  Stage C (N-parallel, 2048 query rows/core): for every head h
     attT = ekT_h-contract-kT in fp8 DoubleRow (2x PE contraction/matmul),
     eatt = exp(svec*attT) bf16 (ACT), out += (eatt^T @ val2_h) / rowsum.
  The final Linear never materializes: x @ Wf^T == sum_h att_h @ (val_h @ Wfh^T),
  and bf is folded into head 0's val2 (attention rows sum to 1).
"""

import numpy as np

H, M, D, O, N = 16, 1024, 512, 512, 16384
NCORES = 8
HPC = H // NCORES          # heads per core
NS = N // NCORES           # query rows per core

EK_SZ = O * M              # ekT elements per head (fp8)
V2_SZ = M * O              # val2 elements per head (bf16)
SV_SZ = M                  # svec elements per head (f32)


def build_nc(ns=NS, rep=1, mock_cc=False):
    """Build + compile the SPMD Bass program (same program on all 8 cores)."""
    from contextlib import ExitStack
    import concourse.tile as tile
    from concourse import bacc, mybir, masks

    f32 = mybir.dt.float32
    fr = mybir.dt.float32r
    b16 = mybir.dt.bfloat16
    f8 = mybir.dt.float8e4
    AF = mybir.ActivationFunctionType
    DR = mybir.MatmulPerfMode.DoubleRow

    OT, DTL, MT = O // 128, D // 128, M // 128      # 4, 4, 8
    NT = ns // 128
    NCH = ns // 512

    nc = bacc.Bacc("TRN2", target_bir_lowering=False, debug=False,
                   num_devices=NCORES)

    k_in = nc.dram_tensor("k", [ns, O], f32, kind="ExternalInput")
    memsT_in = nc.dram_tensor("memsT", [HPC, D, M], fr, kind="ExternalInput")
    wkT_in = nc.dram_tensor("WkT", [HPC, D, O], fr, kind="ExternalInput")
    wvT_in = nc.dram_tensor("WvT", [HPC, D, O], fr, kind="ExternalInput")
    wfT_in = nc.dram_tensor("WfT", [HPC, O, O], fr, kind="ExternalInput")
    bk_in = nc.dram_tensor("bk", [HPC, O], f32, kind="ExternalInput")
    bv_in = nc.dram_tensor("bv", [HPC, O], f32, kind="ExternalInput")
    bf_in = nc.dram_tensor("bf", [HPC, O], fr, kind="ExternalInput")
    out_ext = nc.dram_tensor("out", [ns, O], f32, kind="ExternalOutput")

    def b(ap):  # float32r view for matmul operands
        return ap.bitcast(fr)

    with tile.TileContext(nc, pool_alloc_mode="queue") as tc, ExitStack() as octx:
        dram_pool = octx.enter_context(
            tc.tile_pool(name="dram", bufs=1, space="DRAM"))
        const_pool = octx.enter_context(tc.tile_pool(name="const", bufs=1))
        ident = const_pool.tile([128, 128], f32)
        masks.make_identity(nc, ident[:])
        ones_f32 = const_pool.tile([128, 2], f32)
        nc.gpsimd.memset(ones_f32[:], 1.0)
        ones_b16 = const_pool.tile([128, 2], b16)
        nc.scalar.copy(ones_b16[:], ones_f32[:])
        ones_f8 = const_pool.tile([128, 2], f8)
        nc.scalar.copy(ones_f8[:], ones_f32[:])
        ones_row = const_pool.tile([1, 128], fr)
        ones_row_f32 = const_pool.tile([1, 128], f32)
        nc.gpsimd.memset(ones_row_f32[:], 1.0)
        nc.scalar.copy(ones_row[:], ones_row_f32[:])

        kt_pool = octx.enter_context(tc.tile_pool(name="kt", bufs=1))
        acc_pool = octx.enter_context(tc.tile_pool(name="acc", bufs=1))

        for r in range(rep):
            ek_ins = [dram_pool.tile([EK_SZ], f8, tag=f"ek_in{r}_{j}",
                                     name=f"ek_in{r}_{j}") for j in range(HPC)]
            sv_ins = [dram_pool.tile([SV_SZ], f32, tag=f"sv_in{r}_{j}",
                                     name=f"sv_in{r}_{j}") for j in range(HPC)]
            v2_ins = [dram_pool.tile([V2_SZ], b16, tag=f"v2_in{r}_{j}",
                                     name=f"v2_in{r}_{j}") for j in range(HPC)]
            ek_outs = [dram_pool.tile([NCORES * EK_SZ], f8,
                                      tag=f"ek_out{r}_{j}", name=f"ek_out{r}_{j}",
                                      addr_space="Shared") for j in range(HPC)]
            sv_outs = [dram_pool.tile([NCORES * SV_SZ], f32,
                                      tag=f"sv_out{r}_{j}", name=f"sv_out{r}_{j}",
                                      addr_space="Shared") for j in range(HPC)]
            v2_outs = [dram_pool.tile([NCORES * V2_SZ], b16,
                                      tag=f"v2_out{r}_{j}", name=f"v2_out{r}_{j}",
                                      addr_space="Shared") for j in range(HPC)]

            def ag(src, dst):
                if not mock_cc:
                    nc.gpsimd.collective_compute(
                        "AllGather", mybir.AluOpType.bypass,
                        replica_groups=[list(range(NCORES))],
                        ins=[src[:]], outs=[dst[:]])

            # ============ Stage A: per-local-head key/val precompute ========
            with ExitStack() as actx:
                small = actx.enter_context(tc.tile_pool(name=f"small{r}", bufs=2))
                mm_ps = actx.enter_context(
                    tc.tile_pool(name=f"mm_ps{r}", bufs=3, space="PSUM"))
                ks_ps = actx.enter_context(
                    tc.tile_pool(name=f"ks_ps{r}", bufs=2, space="PSUM"))

                ev_cnt = [0]

                def evac(dst_ap, src_ap):
                    eng = nc.scalar if (ev_cnt[0] % 2 == 0) else nc.vector
                    ev_cnt[0] += 1
                    if eng is nc.scalar:
                        eng.copy(dst_ap, src_ap)
                    else:
                        eng.tensor_copy(dst_ap, src_ap)

                for j in range(HPC):
                    bk_sb = small.tile([128, OT], f32, tag="bk_ld", name="bk_sb")
                    nc.sync.dma_start(
                        bk_sb[:], bk_in[j].rearrange("(t p) -> p t", p=128))
                    bv_sb = small.tile([128, OT], f32, tag="bv_ld", name="bv_sb")
                    nc.sync.dma_start(
                        bv_sb[:], bv_in[j].rearrange("(t p) -> p t", p=128))
                    bf_sb = small.tile([1, O], fr, tag="bf_ld", name="bf_sb")
                    nc.sync.dma_start(
                        bf_sb[:], bf_in[j].rearrange("(a o) -> a o", a=1))

                    memsT, f_memsT = tc.tile([128, DTL, M], fr, name="memsT")
                    nc.sync.dma_start(
                        memsT[:], memsT_in[j].rearrange("(t p) m -> p t m", p=128))
                    wkT, f_wkT = tc.tile([128, DTL, O], fr, name="wkT")
                    nc.sync.dma_start(
                        wkT[:], wkT_in[j].rearrange("(t p) o -> p t o", p=128))
                    wvT, f_wvT = tc.tile([128, DTL, O], fr, name="wvT")
                    nc.sync.dma_start(
                        wvT[:], wvT_in[j].rearrange("(t p) o -> p t o", p=128))
                    wfT, f_wfT = tc.tile([128, OT, O], fr, name="wfT")
                    nc.sync.dma_start(
                        wfT[:], wfT_in[j].rearrange("(t p) o -> p t o", p=128))

                    # ---- unnormalized keys, [o, m] orientation, fp8 payload
                    ek_om, f_ek = tc.tile([128, OT, M], f8, name="ek_om")
                    for ot in range(OT):
                        for mc in range(M // 512):
                            pk = mm_ps.tile([128, 512], f32, tag="mm", name="pk")
                            for dk in range(DTL):
                                nc.tensor.matmul(
                                    pk[:],
                                    wkT[:, dk, ot * 128:(ot + 1) * 128],
                                    memsT[:, dk, mc * 512:(mc + 1) * 512],
                                    start=(dk == 0), stop=(dk == DTL - 1))
                            nc.scalar.activation(
                                ek_om[:, ot, mc * 512:(mc + 1) * 512], pk[:],
                                AF.Exp, bias=bk_sb[:, ot:ot + 1])
                    nc.sync.dma_start(
                        ek_ins[j].rearrange("(ot p m) -> p ot m", ot=OT, p=128),
                        ek_om[:])
                    ag(ek_ins[j], ek_outs[j])

                    # ---- key-softmax normalizer (sum over o = partitions)
                    svec = small.tile([128, MT], f32, tag="svec", name="svec")
                    for ms in range(MT):
                        ks = ks_ps.tile([128, 2], f32, tag="ks", name="ks")
                        for ot in range(OT):
                            nc.tensor.matmul(
                                ks[:],
                                ek_om[:, ot, ms * 128:(ms + 1) * 128],
                                ones_f8[:],
                                start=(ot == 0), stop=(ot == OT - 1))
                        nc.vector.reciprocal(svec[:, ms:ms + 1], ks[:, :1])
                    nc.sync.dma_start(
                        sv_ins[j].rearrange("(p t) -> p t", p=128), svec[:])
                    ag(sv_ins[j], sv_outs[j])

                    # ---- valT [o, m] with bias bv
                    valT, f_valT = tc.tile([128, OT, M], fr, name="valT")
                    for ot in range(OT):
                        for mc in range(M // 512):
                            pv = mm_ps.tile([128, 512], f32, tag="mm", name="pv")
                            for dk in range(DTL):
                                nc.tensor.matmul(
                                    pv[:],
                                    wvT[:, dk, ot * 128:(ot + 1) * 128],
                                    memsT[:, dk, mc * 512:(mc + 1) * 512],
                                    start=(dk == 0), stop=(dk == DTL - 1))
                            nc.scalar.add(
                                valT[:, ot, mc * 512:(mc + 1) * 512], pv[:],
                                bv_sb[:, ot:ot + 1])

                    # ---- val2 [m, oo] = valT^T @ WfT (+ bf), bf16 payload
                    val2, f_val2 = tc.tile([128, MT, O], b16, name="val2")
                    for mt in range(MT):
                        p2 = mm_ps.tile([128, O], f32, tag="mm", name="p2")
                        for ot in range(OT):
                            nc.tensor.matmul(
                                p2[:],
                                valT[:, ot, mt * 128:(mt + 1) * 128],
                                wfT[:, ot, :],
                                start=(ot == 0), stop=False)
                        nc.tensor.matmul(
                            p2[:], ones_row[:1, :], bf_sb[:1, :],
                            start=False, stop=True)
                        evac(val2[:, mt, :], p2[:])
                    nc.sync.dma_start(
                        v2_ins[j].rearrange("(mt p f) -> p mt f", mt=MT, p=128),
                        val2[:])
                    ag(v2_ins[j], v2_outs[j])

                    f_val2()
                    f_valT()
                    f_ek()
                    f_wfT()
                    f_wvT()
                    f_wkT()
                    f_memsT()

            # ============ kT: transpose + fp8-cast this core's k slice ======
            with ExitStack() as tctx:
                kld = tctx.enter_context(tc.tile_pool(name=f"kld{r}", bufs=2))
                tp_ps = tctx.enter_context(
                    tc.tile_pool(name=f"tp_ps{r}", bufs=4, space="PSUM"))
                ev_cnt2 = [0]

                def evac2(dst_ap, src_ap):
                    eng = nc.scalar if (ev_cnt2[0] % 2 == 0) else nc.vector
                    ev_cnt2[0] += 1
                    if eng is nc.scalar:
                        eng.copy(dst_ap, src_ap)
                    else:
                        eng.tensor_copy(dst_ap, src_ap)

                kT = kt_pool.tile([128, OT, ns], f8, tag="kT", name="kT")
                for ng in range(NT // 4):
                    k_sb = kld.tile([128, 4, O], f32, tag="k_ld",
                                    name="k_sb")
                    nc.sync.dma_start(
                        k_sb[:],
                        k_in[ng * 512:(ng + 1) * 512, :].rearrange(
                            "(nt p) o -> p nt o", p=128))
                    for nt in range(4):
                        for ot in range(OT):
                            p = tp_ps.tile([128, 128], f32, tag="tp",
                                           name="tp_ps_t")
                            nc.tensor.transpose(
                                p[:], k_sb[:, nt, ot * 128:(ot + 1) * 128],
                                ident[:])
                            evac2(kT[:, ot,
                                     (ng * 4 + nt) * 128:(ng * 4 + nt + 1) * 128],
                                  p[:])

            # ============ Stage C: attention over all heads ============
            acc = acc_pool.tile([128, NT, O], f32, tag="acc")
            with ExitStack() as cctx:
                h_ld = cctx.enter_context(tc.tile_pool(name=f"h_ld{r}", bufs=2))
                e_sb = cctx.enter_context(tc.tile_pool(name=f"e_sb{r}", bufs=2))
                v_sb = cctx.enter_context(tc.tile_pool(name=f"v_sb{r}", bufs=2))
                att_ps = cctx.enter_context(
                    tc.tile_pool(name=f"att_ps{r}", bufs=4, space="PSUM"))
                o_ps = cctx.enter_context(
                    tc.tile_pool(name=f"o_ps{r}", bufs=2, space="PSUM"))
                rs_ps = cctx.enter_context(
                    tc.tile_pool(name=f"rs_ps{r}", bufs=2, space="PSUM"))

                for hidx in range(H):
                    j, cc = hidx // NCORES, hidx % NCORES
                    if mock_cc:
                        ek_src, sv_src, v2_src = ek_ins[j], sv_ins[j], v2_ins[j]
                        eb = sb = vb = 0
                    else:
                        ek_src, sv_src, v2_src = (ek_outs[j], sv_outs[j],
                                                  v2_outs[j])
                        eb, sb, vb = cc * EK_SZ, cc * SV_SZ, cc * V2_SZ
                    ekt_h = h_ld.tile([128, OT, M], f8, tag="ekt_h")
                    nc.sync.dma_start(
                        ekt_h[:],
                        ek_src[eb:eb + EK_SZ].rearrange(
                            "(ot p m) -> p ot m", ot=OT, p=128))
                    val2_h = h_ld.tile([128, MT, O], b16, tag="val2_h")
                    nc.sync.dma_start(
                        val2_h[:],
                        v2_src[vb:vb + V2_SZ].rearrange(
                            "(mt p f) -> p mt f", mt=MT, p=128))
                    svec_h = h_ld.tile([128, MT], f32, tag="svec_h")
                    nc.sync.dma_start(
                        svec_h[:],
                        sv_src[sb:sb + SV_SZ].rearrange("(p t) -> p t", p=128))

                    for c in range(NCH):
                        eatt = e_sb.tile([128, MT, 512], b16, tag="eatt")
                        for mt in range(MT):
                            pa = att_ps.tile([128, 512], f32, tag="att")
                            for t2 in range(OT // 2):
                                nc.tensor.matmul(
                                    pa[:],
                                    ekt_h[:, 2 * t2:2 * t2 + 2,
                                          mt * 128:(mt + 1) * 128],
                                    kT[:, 2 * t2:2 * t2 + 2,
                                       c * 512:(c + 1) * 512],
                                    start=(t2 == 0), stop=(t2 == OT // 2 - 1),
                                    perf_mode=DR)
                            nc.scalar.activation(
                                eatt[:, mt, :], pa[:], AF.Exp,
                                scale=svec_h[:, mt:mt + 1])
                        for nt in range(4):
                            po = o_ps.tile([128, O], f32, tag="o")
                            prs = rs_ps.tile([128, 2], f32, tag="rs")
                            for mt in range(MT):
                                nc.tensor.matmul(
                                    po[:],
                                    eatt[:, mt, nt * 128:(nt + 1) * 128],
                                    val2_h[:, mt, :],
                                    start=(mt == 0), stop=(mt == MT - 1))
                            for mt in range(MT):
                                nc.tensor.matmul(
                                    prs[:],
                                    eatt[:, mt, nt * 128:(nt + 1) * 128],
                                    ones_b16[:],
                                    start=(mt == 0), stop=(mt == MT - 1))
                            rec = v_sb.tile([128, 1], f32, tag="rec")
                            nc.vector.reciprocal(rec[:], prs[:, :1])
                            gnt = c * 4 + nt
                            if hidx == 0:
                                nc.vector.tensor_scalar_mul(
                                    acc[:, gnt, :], po[:], rec[:, :1])
                            else:
                                tmp = v_sb.tile([128, O], f32, tag="tmp")
                                nc.vector.tensor_scalar_mul(
                                    tmp[:], po[:], rec[:, :1])
                                nc.vector.tensor_add(
                                    acc[:, gnt, :], acc[:, gnt, :], tmp[:])

            nc.sync.dma_start(
                out_ext[:, :].rearrange("(nt p) o -> p nt o", p=128), acc[:])

    nc.compile()
    return nc


# ----------------------------------------------------------------------------
# Host-side execution: persistent jitted 8-core dispatch (axon/PJRT).
# ----------------------------------------------------------------------------
_EXEC_CACHE = {}


def _get_exec(ns=NS, rep=1):
    key = (ns, rep)
    if key in _EXEC_CACHE:
        return _EXEC_CACHE[key]

    import jax
    import numpy as _np
    from jax.sharding import Mesh, PartitionSpec
    from jax.experimental.shard_map import shard_map
    from concourse import mybir
    from concourse.bass2jax import (_bass_exec_p, install_neuronx_cc_hook,
                                    partition_id_tensor)

    nc = build_nc(ns=ns, rep=rep)
    # surface walrus/compile errors (PJRT swallows python hook exceptions)
    from concourse import bass2jax as _b2j
    if not getattr(_b2j, "_hook_wrapped", False):
        _orig = _b2j.neuronx_cc_hook

        def _wrapped(*a, **kw):
            try:
                return _orig(*a, **kw)
            except BaseException:
                import traceback
                traceback.print_exc()
                raise
        _b2j.neuronx_cc_hook = _wrapped
        _b2j._hook_wrapped = True
    install_neuronx_cc_hook()

    partition_name = (nc.partition_id_tensor.name
                      if nc.partition_id_tensor else None)
    in_names, out_names, out_avals, zero_outs = [], [], [], []
    for alloc in nc.m.functions[0].allocations:
        if not isinstance(alloc, mybir.MemoryLocationSet):
            continue
        name = alloc.memorylocations[0].name
        if alloc.kind == "ExternalInput":
            if name != partition_name:
                in_names.append(name)
        elif alloc.kind == "ExternalOutput":
            out_names.append(name)
            out_avals.append(jax.core.ShapedArray(
                tuple(alloc.tensor_shape), mybir.dt.np(alloc.dtype)))
            zero_outs.append(_np.zeros(tuple(alloc.tensor_shape),
                                       mybir.dt.np(alloc.dtype)))
    names_all = list(in_names) + list(out_names)
    if partition_name is not None:
        names_all.append(partition_name)

    def _body(*args):
        operands = list(args)
        if partition_name is not None:
            operands.append(partition_id_tensor())
        return tuple(_bass_exec_p.bind(
            *operands, out_avals=tuple(out_avals), in_names=tuple(names_all),
            out_names=tuple(out_names), lowering_input_output_aliases=(),
            sim_require_finite=True, sim_require_nnan=True, nc=nc))

    devices = jax.devices()[:NCORES]
    mesh = Mesh(_np.asarray(devices), ("core",))
    n_args = len(in_names) + len(out_names)
    fn = jax.jit(
        shard_map(_body, mesh=mesh,
                  in_specs=(PartitionSpec("core"),) * n_args,
                  out_specs=(PartitionSpec("core"),) * len(out_names),
                  check_rep=False),
        keep_unused=True)

    exec_info = {
        "fn": fn, "in_names": in_names, "out_names": out_names,
        "zero_outs": zero_outs, "nc": nc, "mesh": mesh,
    }
    _EXEC_CACHE[key] = exec_info
    return exec_info


def make_in_maps(k, mems, Wk, bk, Wv, bv, Wf, bf):
    """Shard full inputs into per-core input dicts (host-side layout prep)."""
    c32 = lambda x: np.ascontiguousarray(np.asarray(x, dtype=np.float32))
    k, mems, Wk, bk, Wv, bv, Wf, bf = map(c32, (k, mems, Wk, bk, Wv, bv, Wf, bf))
    in_maps = []
    for r in range(NCORES):
        h0 = r * HPC
        memsT = np.stack([np.ascontiguousarray(mems[h0 + j].T)
                          for j in range(HPC)])
        wkT = np.stack([np.ascontiguousarray(Wk[h0 + j].T)
                        for j in range(HPC)])
        wvT = np.stack([np.ascontiguousarray(Wv[h0 + j].T)
                        for j in range(HPC)])
        wfT = np.stack([
            np.ascontiguousarray(Wf[:, (h0 + j) * O:(h0 + j + 1) * O].T)
            for j in range(HPC)])
        bf_eff = np.zeros((HPC, O), dtype=np.float32)
        if r == 0:
            bf_eff[0] = bf
        in_maps.append({
            "k": k[r * NS:(r + 1) * NS],
            "memsT": memsT,
            "WkT": wkT, "bk": bk[h0:h0 + HPC] - 2.0,
            "WvT": wvT, "bv": bv[h0:h0 + HPC],
            "WfT": wfT, "bf": bf_eff,
        })
    return in_maps


def run_on_hw(in_maps, rep=1):
    """Run the SPMD program; returns full [N, O] output."""
    import jax
    import jax.numpy as jnp
    from jax.sharding import NamedSharding, PartitionSpec
    ex = _get_exec(ns=NS, rep=rep)
    sh = NamedSharding(ex["mesh"], PartitionSpec("core"))
    args = [
        jax.device_put(np.concatenate([m[name] for m in in_maps], axis=0), sh)
        for name in ex["in_names"]]
    zeros = [
        jnp.zeros((NCORES * z.shape[0], *z.shape[1:]), z.dtype,
                  device=sh)
        for z in ex["zero_outs"]]
    outs = ex["fn"](*args, *zeros)
    out = np.asarray(outs[ex["out_names"].index("out")])
    return out


def kernel(**inputs):
    in_maps = make_in_maps(
        inputs["k"], inputs["mems"], inputs["Wk"], inputs["bk"],
        inputs["Wv"], inputs["bv"], inputs["Wf"], inputs["bf"])
    return run_on_hw(in_maps, rep=1)
